# revision 49
# baseline (speedup 1.0000x reference)
"""H2GCN forward on 8 Trainium2 NeuronCores.

out = concat([h0, A1@h0, A2@h0], 1) @ W_out + b_out,  h0 = x @ W1

Data-parallel over destination nodes (1250 rows/core).  Per core:
h0 = x@W1 in bf16 (row-tile pipelined with the x DMA), h0 quantized to
fp8-e4m3 and AllGathered in five 2-tile chunks so the SpMM can start as
soon as the first chunk lands.  SpMM as dense fp8 DoubleRow matmuls with
the adjacency blocks as the MOVING operand and h0 tiles stationary; each
stationary load is shared across the three dest-chunk matmuls via
non-self-loading matmuls (ldweights=False) chained with explicit deps so
the scheduler cannot interleave another weight load.  A1 is pre-scaled
by 16 and A2 by 32 (undone in W_out rows) to keep edge weights in fp8's
normal range.  The output GEMM is split in three k-stages accumulated in
SBUF (bias+h0 during the AllGather gap, the h1 part after the A1 pass,
h2 in the tail) and written once as bf16.
"""
import sys
import types

for _p in ("/opt/trn_rl_repo", "/root/.axon_site", "/root/.axon_site/_ro/trn_rl_repo",
           "/root/.axon_site/_ro/pypackages"):
    if _p not in sys.path:
        sys.path.append(_p)

import numpy as np
import ml_dtypes
import concourse.bass as bass
import concourse.bacc as bacc
import concourse.mybir as mybir
import concourse.tile as tile
from concourse import bass_utils
from concourse.tile import add_dep_helper

N, IN_C, HID, OUT_C = 10000, 2048, 256, 256
NCORES = 8
ROWS = N // NCORES          # 1250
PROWS = 1280                # padded (10 x 128) source rows
NT = PROWS // 128           # 10 row tiles per core
KT = IN_C // 128            # 16 k tiles
ST = NCORES * NT            # 80 source tiles in the padded gather space
NG = ST // 2                # 40 source super-tiles (DoubleRow pairs)
DCOLS = ROWS                # dest columns (no padding in A)
# AllGather chunks (h0 tile ranges).  Every collective costs ~12us on the
# serialized CC stream after a fixed runtime barrier (~76us wall), so use
# a small first chunk to start the SpMM early, then two medium ones.
CH = [(0, 8), (8, 10)]
NCH = len(CH)
DCH = (512, 512, DCOLS - 1024)   # dest psum chunks
# spmm stream order: supers grouped by AllGather chunk (chunk-major);
# super s of core r covers h0 tiles (2s, 2s+1) of core r.
G_ORDER = [r * (NT // 2) + s
           for lo, hi in CH
           for s in range(lo // 2, hi // 2)
           for r in range(NCORES)]

f32 = mybir.dt.float32
bf16 = mybir.dt.bfloat16
f8 = mybir.dt.float8e4
bfnp = ml_dtypes.bfloat16
f8np = ml_dtypes.float8_e4m3fn

# blob_a (bf16): W1 k-tiles then x row-tile-major k-tiles
OW1, OX = 0, KT * HID
BLOBA = KT * HID + NT * KT * 128
# blob_b (bf16): Wout k-tiles, bias (row 0), ones (row 0), identity
OWO, OB, OO, OI = 0, 6 * OUT_C, 6 * OUT_C + OUT_C, 6 * OUT_C + OUT_C + 128
BLOBB = OI + 128

LAST_EXEC_NS = None
LAST_RESULTS = None


def _install_trace_shim():
    try:
        import antenv.axon_hooks  # noqa: F401
        return
    except ImportError:
        pass
    try:
        import antenv
        from trn_agent_boot.trn_boot import _ntff_profile_via_ctypes
        hook = _ntff_profile_via_ctypes("/opt/axon/libaxon_pjrt.so")
        mod = types.ModuleType("antenv.axon_hooks")
        mod.get_axon_ntff_profile_hook = lambda: hook
        mod.set_axon_ntff_profile_hook = lambda h: None
        sys.modules["antenv.axon_hooks"] = mod
        antenv.axon_hooks = mod
    except Exception:
        pass


def _pack_adj(rows, cols, vals, core, scale):
    """fp8 dense A^T for this core's dest shard, laid out
    [128 src-part, NG super (stream order), 2 ktile, DCOLS dest]."""
    lo, hi = core * ROWS, (core + 1) * ROWS
    m = (rows >= lo) & (rows < hi)
    r, c, v = rows[m] - lo, cols[m], vals[m] * scale
    A = np.zeros((NCORES * PROWS, DCOLS), np.float32)
    src = (c // ROWS) * PROWS + (c % ROWS)
    np.add.at(A, (src, r), v)
    A4 = A.reshape(NG, 2, 128, DCOLS)[G_ORDER]
    return np.ascontiguousarray(
        A4.transpose(2, 0, 1, 3).reshape(128, NG * 2 * DCOLS)).astype(f8np)


def _dedup_ldweights(nc):
    """Remove InstLdweights that reload the PE array with the exact weights
    the previous load (in PE queue order) already holds.  The Tile lowering
    splits every matmul into ldweights+matmul; for the SpMM the three
    dest-chunk matmuls of a group share one stationary tile, so two of the
    three loads are redundant (and in fp8 DoubleRow mode the load cannot
    hide under the matmul, costing ~107ns each).  Waits on a removed load
    are carried onto the next retained PE instruction; loads carrying
    semaphore updates are never removed."""
    removed = 0
    for blk in nc.main_func.blocks:
        insts = list(blk.instructions)
        cur_sig = None
        pending = []
        keep = []
        pe_engine = None
        for i in insts:
            if isinstance(i, (mybir.InstLdweights, mybir.InstMatmult)):
                pe_engine = i.engine
                break
        for i in insts:
            if getattr(i, "engine", None) != pe_engine:
                keep.append(i)
                continue
            if isinstance(i, mybir.InstLdweights):
                si = i.sync_info
                sig = (str(i.ins[0]), str(i.perf_mode), str(i.is_transpose),
                       str(i.tile_position), str(i.tile_size))
                if sig == cur_sig and not (si is not None and si.on_update):
                    if si is not None and si.on_wait:
                        pending.extend(si.on_wait)
                    removed += 1
                    continue
                cur_sig = sig
            elif not isinstance(i, mybir.InstMatmult):
                # any other PE instruction invalidates the loaded weights
                cur_sig = None
            if pending:
                si = i.sync_info
                if si is None:
                    i.sync_info = mybir.SyncInfo(on_wait=list(pending),
                                                 on_update=[])
                else:
                    merged = list(si.on_wait)
                    merged.extend(pending)
                    i.sync_info = mybir.SyncInfo(on_wait=merged,
                                                 on_update=list(si.on_update))
                pending = []
            keep.append(i)
        assert not pending
        if removed and len(keep) != len(insts):
            blk.instructions = keep
    return removed


def _build():
    nc = bacc.Bacc("TRN2", target_bir_lowering=False, debug=False,
                   num_devices=8)
    blob_a = nc.dram_tensor("blob_a", [128, BLOBA], bf16, kind="ExternalInput")
    blob_b = nc.dram_tensor("blob_b", [128, BLOBB], bf16, kind="ExternalInput")
    A_d = nc.dram_tensor("A_d", [128, 2 * NG * 2 * DCOLS], f8,
                         kind="ExternalInput")
    out = nc.dram_tensor("out", [ROWS, OUT_C], bf16, kind="ExternalOutput")

    DR = mybir.MatmulPerfMode.DoubleRow

    with tile.TileContext(nc) as tc:
        with tc.tile_pool(name="keep", bufs=1) as keep, \
             tc.tile_pool(name="dram", bufs=1, space="DRAM") as dram, \
             tc.tile_pool(name="pmm", bufs=2, space="PSUM") as pmm, \
             tc.tile_pool(name="pss", bufs=1, space="PSUM") as pss:

            ag_sb = keep.tile([128, NT, HID], f8)       # h0 fp8 (AG staging)
            h0a = keep.tile([128, NCORES, NT, HID], f8)  # gathered global h0
            hT = keep.tile([128, 6, PROWS], bf16)       # feature-major concat
            o_sb = keep.tile([128, NT, OUT_C], f32)     # out accumulator
            wout_sb = keep.tile([128, BLOBB], bf16)
            pa_t = keep.tile([128, BLOBA], bf16)

            nc.sync.dma_start(wout_sb[:], blob_b[:])
            ident = wout_sb[:, OI:OI + 128]

            ag_ins, ag_outs = [], []
            for ci, (lo, hi) in enumerate(CH):
                w = hi - lo
                ag_ins.append(dram.tile([128, w * HID], f8,
                                        name=f"ag_in{ci}"))
                ag_outs.append(dram.tile([NCORES * 128, w * HID], f8,
                                         addr_space="Shared",
                                         name=f"ag_out{ci}"))

            # ---- phase A: h0 = x @ W1 (bf16), row-tile pipelined ----
            # A_d streams on the scalar queue; x + small DMAs on sync.
            last_pa_inst = None
            with nc.named_scope("h0_gemm"):
                nc.sync.dma_start(pa_t[:, OW1:OW1 + KT * HID],
                                  blob_a[:, OW1:OW1 + KT * HID])
                for t in range(NT):
                    o = OX + t * KT * 128
                    if t < 3:
                        nc.sync.dma_start(pa_t[:, o:o + KT * 128],
                                          blob_a[:, o:o + KT * 128])
                for t in range(NT):
                    if t + 3 < NT:
                        o2 = OX + (t + 3) * KT * 128
                        nc.sync.dma_start(pa_t[:, o2:o2 + KT * 128],
                                          blob_a[:, o2:o2 + KT * 128])
                    ps = pmm.tile([128, HID], f32, tag="mm")
                    o = OX + t * KT * 128
                    for k in range(KT):
                        nc.tensor.matmul(
                            ps[:],
                            pa_t[:, o + k * 128:o + (k + 1) * 128],
                            pa_t[:, OW1 + k * HID:OW1 + (k + 1) * HID],
                            start=(k == 0), stop=(k == KT - 1),
                        )
                    h = keep.tile([128, HID], bf16, tag="hsb", bufs=3)
                    nc.vector.tensor_copy(h[:], ps[:])
                    nc.vector.tensor_copy(ag_sb[:, t, :], ps[:])
                    for half in range(2):
                        pt = pmm.tile([128, HID], f32, tag="mm")
                        ptb = pt[:].bitcast(bf16)[:, 0:128]
                        last_pa_inst = nc.tensor.transpose(
                            ptb, h[:, 128 * half:128 * (half + 1)], ident)
                        nc.vector.tensor_copy(
                            hT[:, half, 128 * t:128 * (t + 1)], ptb)
                    for ci, (lo, hi) in enumerate(CH):
                        if t == hi - 1:
                            nc.gpsimd.dma_start(
                                ag_ins[ci][:].rearrange(
                                    "p (a m) -> p a m", a=hi - lo),
                                ag_sb[:, lo:hi, :])

            # ---- phase B: AllGather h0 (fp8) in 5 chunks, then unpack ----
            with nc.named_scope("allgather"):
                for ci in range(NCH):
                    nc.gpsimd.collective_compute(
                        "AllGather", mybir.AluOpType.bypass,
                        replica_groups=[list(range(NCORES))],
                        ins=[ag_ins[ci].opt()], outs=[ag_outs[ci].opt()],
                    )
                # unpack per 2-tile super group so the SpMM can start on the
                # first supers while the rest of the chunk unpacks
                for ci, (lo, hi) in enumerate(CH):
                    for s in range(lo // 2, hi // 2):
                        o = (2 * s - lo) * HID
                        nc.sync.dma_start(
                            h0a[:, :, 2 * s:2 * s + 2, :],
                            ag_outs[ci][:, o:o + 2 * HID].rearrange(
                                "(r p) (t m) -> p r t m", p=128, t=2))

            # ---- phase D1: out += bias + h0-part (runs in the AG gap) ----
            last_d1_inst = None
            with nc.named_scope("out_gemm"):
                for t in range(NT):
                    ps = pmm.tile([128, OUT_C], f32, tag="mm")
                    nc.tensor.matmul(ps[:], wout_sb[0:1, OO:OO + 128],
                                     wout_sb[0:1, OB:OB + OUT_C],
                                     start=True, stop=False)
                    for k in range(2):
                        last_d1_inst = nc.tensor.matmul(
                            ps[:],
                            hT[:, k, 128 * t:128 * (t + 1)],
                            wout_sb[:, OWO + k * OUT_C:OWO + (k + 1) * OUT_C],
                            start=False, stop=(k == 1),
                        )
                    nc.vector.tensor_copy(o_sb[:, t, :], ps[:])

            # ---- phase C: SpMM, fp8 DoubleRow, A moving / h0 stationary ----
            # out[feat, dest] += sum_src h0a[src, feat] * A[src, dest]
            # One ldweights per (super, feat-half) shared by the 3 dest-chunk
            # matmuls x 1 adjacency; the whole PE stream is chained so no
            # other weight load can slip inside a reuse group.
            chain_prev = [None]

            def chain(bi, extra=None):
                if chain_prev[0] is not None:
                    add_dep_helper(bi.ins, chain_prev[0].ins,
                                   reason="pe weight-reuse chain")
                if extra is not None:
                    add_dep_helper(bi.ins, extra.ins, reason="pe chain head")
                chain_prev[0] = bi

            with nc.named_scope("spmm"):
                for a in range(2):
                    ps6 = pss.tile([128, 2, 3, 512], f32, tag="spmm",
                                   name=f"ps_spmm{a}")
                    for p in range(NG):
                        g = G_ORDER[p]
                        r, tl = g // (NT // 2), 2 * (g % (NT // 2))
                        at = keep.tile([128, 2, DCOLS], f8, tag="a", bufs=12)
                        off = (a * NG + p) * 2 * DCOLS
                        nc.scalar.dma_start(
                            at[:], A_d[:, off:off + 2 * DCOLS]
                            .rearrange("p (a m) -> p a m", a=2))
                        for fh in range(2):
                            lhs = h0a[:, r, tl:tl + 2,
                                      128 * fh:128 * (fh + 1)]
                            head = (a == 0 and p == 0 and fh == 0)
                            dpos = 0
                            for d, w in enumerate(DCH):
                                mm = nc.tensor.matmul(
                                    ps6[:, fh, d, 0:w], lhs,
                                    at[:, :, dpos:dpos + w],
                                    start=(p == 0), stop=(p == NG - 1),
                                    perf_mode=DR,
                                )
                                # head of chain waits for all pre-spmm PE work
                                chain(mm, extra=last_pa_inst
                                      if (head and d == 0) else None)
                                if head and d == 0:
                                    add_dep_helper(mm.ins, last_d1_inst.ins,
                                                   reason="pe chain head d1")
                                dpos += w
                    nc.vector.tensor_copy(
                        hT[:, 2 + 2 * a, 0:DCOLS],
                        ps6[:, 0].rearrange("p a b -> p (a b)")[:, 0:DCOLS])
                    nc.scalar.copy(
                        hT[:, 2 + 2 * a + 1, 0:DCOLS],
                        ps6[:, 1].rearrange("p a b -> p (a b)")[:, 0:DCOLS])
                    if a == 0:
                        # ---- phase D2: out += h1-part (chained into the
                        # PE stream between the two adjacency passes) ----
                        with nc.named_scope("out_gemm"):
                            for t in range(NT):
                                rows = min(128, ROWS - 128 * t)
                                ps = pmm.tile([128, OUT_C], f32, tag="mm")
                                for k in range(2, 4):
                                    mmo = nc.tensor.matmul(
                                        ps[:rows],
                                        hT[:, k, 128 * t:128 * t + rows],
                                        wout_sb[:, OWO + k * OUT_C:
                                                OWO + (k + 1) * OUT_C],
                                        start=(k == 2), stop=(k == 3),
                                    )
                                    chain(mmo)
                                nc.vector.tensor_add(
                                    o_sb[:rows, t, :], o_sb[:rows, t, :],
                                    ps[:rows])

            # ---- phase D3: out += h2-part, cast bf16, single DMA out ----
            with nc.named_scope("out_gemm"):
                for t in range(NT):
                    rows = min(128, ROWS - 128 * t)
                    ps = pmm.tile([128, OUT_C], f32, tag="mm")
                    for k in range(4, 6):
                        nc.tensor.matmul(
                            ps[:rows],
                            hT[:, k, 128 * t:128 * t + rows],
                            wout_sb[:, OWO + k * OUT_C:OWO + (k + 1) * OUT_C],
                            start=(k == 4), stop=(k == 5),
                        )
                    ob = keep.tile([128, OUT_C], bf16, tag="ob", bufs=4)
                    nc.vector.tensor_add(ob[:rows], ps[:rows],
                                         o_sb[:rows, t, :])
                    nc.sync.dma_start(out[128 * t:128 * t + rows, :],
                                      ob[:rows, :])
    _dedup_ldweights(nc)
    nc.compile()
    return nc


def kernel(x, adj1_rows, adj1_cols, adj1_vals, adj2_rows, adj2_cols, adj2_vals,
           W1, W_out, b_out):
    global LAST_EXEC_NS, LAST_RESULTS
    _install_trace_shim()
    x = np.asarray(x, np.float32)
    W1 = np.ascontiguousarray(np.asarray(W1, np.float32))
    W_out = np.ascontiguousarray(np.asarray(W_out, np.float32)).copy()
    b_out = np.asarray(b_out, np.float32).ravel()
    # undo the fp8-range pre-scaling of A1/A2 in the matching W_out rows
    W_out[HID:2 * HID] *= 1.0 / 16.0
    W_out[2 * HID:3 * HID] *= 1.0 / 32.0

    w1_cols = W1.reshape(KT, 128, HID).transpose(1, 0, 2).reshape(128, KT * HID)
    blob_b = np.zeros((128, BLOBB), np.float32)
    blob_b[:, OWO:OWO + 6 * OUT_C] = \
        W_out.reshape(6, 128, OUT_C).transpose(1, 0, 2).reshape(128, 6 * OUT_C)
    blob_b[0, OB:OB + OUT_C] = b_out
    blob_b[0, OO:OO + 128] = 1.0
    blob_b[:, OI:OI + 128] = np.eye(128, dtype=np.float32)
    blob_b = blob_b.astype(bfnp)

    a1r = np.asarray(adj1_rows, np.int64)
    a1c = np.asarray(adj1_cols, np.int64)
    a1v = np.asarray(adj1_vals, np.float32)
    a2r = np.asarray(adj2_rows, np.int64)
    a2c = np.asarray(adj2_cols, np.int64)
    a2v = np.asarray(adj2_vals, np.float32)

    in_maps = []
    for c in range(NCORES):
        xtp = np.zeros((IN_C, PROWS), np.float32)
        xtp[:, :ROWS] = x[c * ROWS:(c + 1) * ROWS].T
        blob_a = np.concatenate([
            w1_cols,
            xtp.reshape(KT, 128, NT, 128).transpose(1, 2, 0, 3)
            .reshape(128, NT * KT * 128),
        ], axis=1).astype(bfnp)
        A_pack = np.concatenate([
            _pack_adj(a1r, a1c, a1v, c, 16.0),
            _pack_adj(a2r, a2c, a2v, c, 32.0),
        ], axis=1)
        in_maps.append({"blob_a": blob_a, "blob_b": blob_b, "A_d": A_pack})

    nc = _build()
    try:
        res = bass_utils.run_bass_kernel_spmd(
            nc, in_maps, core_ids=list(range(NCORES)), trace=True,
            trace_cores=[0])
    except Exception:
        res = bass_utils.run_bass_kernel_spmd(
            nc, in_maps, core_ids=list(range(NCORES)), trace=False)
    LAST_EXEC_NS = res.exec_time_ns
    LAST_RESULTS = res
    return np.concatenate(
        [np.asarray(res.results[c]["out"]).astype(np.float32)
         for c in range(NCORES)], axis=0)


# revision 50
# speedup vs baseline: 1.0695x; 1.0695x over previous
"""H2GCN forward on 8 Trainium2 NeuronCores.

out = concat([h0, A1@h0, A2@h0], 1) @ W_out + b_out,  h0 = x @ W1

Data-parallel over destination nodes (1250 rows/core).  Per core:
h0 = x@W1 in bf16 (row-tile pipelined with the x DMA), h0 quantized to
fp8-e4m3 and AllGathered in five 2-tile chunks so the SpMM can start as
soon as the first chunk lands.  SpMM as dense fp8 DoubleRow matmuls with
the adjacency blocks as the MOVING operand and h0 tiles stationary; each
stationary load is shared across the three dest-chunk matmuls via
non-self-loading matmuls (ldweights=False) chained with explicit deps so
the scheduler cannot interleave another weight load.  A1 is pre-scaled
by 16 and A2 by 32 (undone in W_out rows) to keep edge weights in fp8's
normal range.  The output GEMM is split in three k-stages accumulated in
SBUF (bias+h0 during the AllGather gap, the h1 part after the A1 pass,
h2 in the tail) and written once as bf16.
"""
import sys
import types

for _p in ("/opt/trn_rl_repo", "/root/.axon_site", "/root/.axon_site/_ro/trn_rl_repo",
           "/root/.axon_site/_ro/pypackages"):
    if _p not in sys.path:
        sys.path.append(_p)

import numpy as np
import ml_dtypes
import concourse.bass as bass
import concourse.bacc as bacc
import concourse.mybir as mybir
import concourse.tile as tile
from concourse import bass_utils
from concourse.tile import add_dep_helper

N, IN_C, HID, OUT_C = 10000, 2048, 256, 256
NCORES = 8
ROWS = N // NCORES          # 1250
PROWS = 1280                # padded (10 x 128) source rows
NT = PROWS // 128           # 10 row tiles per core
KT = IN_C // 128            # 16 k tiles
ST = NCORES * NT            # 80 source tiles in the padded gather space
NG = ST // 2                # 40 source super-tiles (DoubleRow pairs)
DCOLS = ROWS                # dest columns (no padding in A)
# AllGather chunks (h0 tile ranges).  Every collective costs ~12us on the
# serialized CC stream after a fixed runtime barrier (~76us wall), so use
# a small first chunk to start the SpMM early, then two medium ones.
CH = [(0, 8), (8, 10)]
NCH = len(CH)
DCH = (512, 512, DCOLS - 1024)   # dest psum chunks
# spmm stream order: supers grouped by AllGather chunk (chunk-major);
# super s of core r covers h0 tiles (2s, 2s+1) of core r.
G_ORDER = [r * (NT // 2) + s
           for lo, hi in CH
           for s in range(lo // 2, hi // 2)
           for r in range(NCORES)]

f32 = mybir.dt.float32
bf16 = mybir.dt.bfloat16
f8 = mybir.dt.float8e4
bfnp = ml_dtypes.bfloat16
f8np = ml_dtypes.float8_e4m3fn

# blob_a (bf16): W1 k-tiles then x row-tile-major k-tiles
OW1, OX = 0, KT * HID
BLOBA = KT * HID + NT * KT * 128
# blob_b (bf16): Wout k-tiles, bias (row 0), ones (row 0), identity
OWO, OB, OO, OI = 0, 6 * OUT_C, 6 * OUT_C + OUT_C, 6 * OUT_C + OUT_C + 128
BLOBB = OI + 128

LAST_EXEC_NS = None
LAST_RESULTS = None


def _install_trace_shim():
    try:
        import antenv.axon_hooks  # noqa: F401
        return
    except ImportError:
        pass
    try:
        import antenv
        from trn_agent_boot.trn_boot import _ntff_profile_via_ctypes
        hook = _ntff_profile_via_ctypes("/opt/axon/libaxon_pjrt.so")
        mod = types.ModuleType("antenv.axon_hooks")
        mod.get_axon_ntff_profile_hook = lambda: hook
        mod.set_axon_ntff_profile_hook = lambda h: None
        sys.modules["antenv.axon_hooks"] = mod
        antenv.axon_hooks = mod
    except Exception:
        pass


def _pack_adj(rows, cols, vals, core, scale):
    """fp8 dense A^T for this core's dest shard, laid out
    [128 src-part, NG super (stream order), 2 ktile, DCOLS dest]."""
    lo, hi = core * ROWS, (core + 1) * ROWS
    m = (rows >= lo) & (rows < hi)
    r, c, v = rows[m] - lo, cols[m], vals[m] * scale
    A = np.zeros((NCORES * PROWS, DCOLS), np.float32)
    src = (c // ROWS) * PROWS + (c % ROWS)
    np.add.at(A, (src, r), v)
    A4 = A.reshape(NG, 2, 128, DCOLS)[G_ORDER]
    return np.ascontiguousarray(
        A4.transpose(2, 0, 1, 3).reshape(128, NG * 2 * DCOLS)).astype(f8np)


def _dedup_ldweights(nc):
    """Remove InstLdweights that reload the PE array with the exact weights
    the previous load (in PE queue order) already holds.  The Tile lowering
    splits every matmul into ldweights+matmul; for the SpMM the three
    dest-chunk matmuls of a group share one stationary tile, so two of the
    three loads are redundant (and in fp8 DoubleRow mode the load cannot
    hide under the matmul, costing ~107ns each).  Waits on a removed load
    are carried onto the next retained PE instruction; loads carrying
    semaphore updates are never removed."""
    removed = 0
    for blk in nc.main_func.blocks:
        insts = list(blk.instructions)
        cur_sig = None
        pending = []
        keep = []
        pe_engine = None
        for i in insts:
            if isinstance(i, (mybir.InstLdweights, mybir.InstMatmult)):
                pe_engine = i.engine
                break
        for i in insts:
            if getattr(i, "engine", None) != pe_engine:
                keep.append(i)
                continue
            if isinstance(i, mybir.InstLdweights):
                si = i.sync_info
                sig = (str(i.ins[0]), str(i.perf_mode), str(i.is_transpose),
                       str(i.tile_position), str(i.tile_size))
                if sig == cur_sig and not (si is not None and si.on_update):
                    if si is not None and si.on_wait:
                        pending.extend(si.on_wait)
                    removed += 1
                    continue
                cur_sig = sig
            elif not isinstance(i, mybir.InstMatmult):
                # any other PE instruction invalidates the loaded weights
                cur_sig = None
            if pending:
                si = i.sync_info
                if si is None:
                    i.sync_info = mybir.SyncInfo(on_wait=list(pending),
                                                 on_update=[])
                else:
                    merged = list(si.on_wait)
                    merged.extend(pending)
                    i.sync_info = mybir.SyncInfo(on_wait=merged,
                                                 on_update=list(si.on_update))
                pending = []
            keep.append(i)
        assert not pending
        if removed and len(keep) != len(insts):
            blk.instructions = keep
    return removed


def _build():
    nc = bacc.Bacc("TRN2", target_bir_lowering=False, debug=False,
                   num_devices=8)
    blob_a = nc.dram_tensor("blob_a", [128, BLOBA], bf16, kind="ExternalInput")
    blob_b = nc.dram_tensor("blob_b", [128, BLOBB], bf16, kind="ExternalInput")
    A_d = nc.dram_tensor("A_d", [128, 2 * NG * 2 * DCOLS], f8,
                         kind="ExternalInput")
    out = nc.dram_tensor("out", [ROWS, OUT_C], bf16, kind="ExternalOutput")

    DR = mybir.MatmulPerfMode.DoubleRow

    with tile.TileContext(nc) as tc:
        with tc.tile_pool(name="keep", bufs=1) as keep, \
             tc.tile_pool(name="dram", bufs=1, space="DRAM") as dram, \
             tc.tile_pool(name="pmm", bufs=2, space="PSUM") as pmm, \
             tc.tile_pool(name="pss", bufs=1, space="PSUM") as pss:

            ag_sb = keep.tile([128, NT, HID], f8)       # h0 fp8 (AG staging)
            h0a = keep.tile([128, NCORES, NT, HID], f8)  # gathered global h0
            hT = keep.tile([128, 6, PROWS], bf16)       # feature-major concat
            o_sb = keep.tile([128, NT, OUT_C], f32)     # out accumulator
            wout_sb = keep.tile([128, BLOBB], bf16)
            pa_t = keep.tile([128, BLOBA], bf16)

            nc.sync.dma_start(wout_sb[:], blob_b[:])
            ident = wout_sb[:, OI:OI + 128]

            ag_ins, ag_outs = [], []
            for ci, (lo, hi) in enumerate(CH):
                w = hi - lo
                ag_ins.append(dram.tile([128, w * HID], f8,
                                        name=f"ag_in{ci}"))
                ag_outs.append(dram.tile([NCORES * 128, w * HID], f8,
                                         addr_space="Shared",
                                         name=f"ag_out{ci}"))

            # ---- phase A: h0 = x @ W1 (bf16), row-tile pipelined ----
            # A_d streams on the scalar queue; x + small DMAs on sync.
            last_pa_inst = None
            with nc.named_scope("h0_gemm"):
                nc.sync.dma_start(pa_t[:, OW1:OW1 + KT * HID],
                                  blob_a[:, OW1:OW1 + KT * HID])
                for t in range(NT):
                    o = OX + t * KT * 128
                    if t < 3:
                        nc.sync.dma_start(pa_t[:, o:o + KT * 128],
                                          blob_a[:, o:o + KT * 128])
                for t in range(NT):
                    if t + 3 < NT:
                        o2 = OX + (t + 3) * KT * 128
                        nc.sync.dma_start(pa_t[:, o2:o2 + KT * 128],
                                          blob_a[:, o2:o2 + KT * 128])
                    ps = pmm.tile([128, HID], f32, tag="mm")
                    o = OX + t * KT * 128
                    for k in range(KT):
                        nc.tensor.matmul(
                            ps[:],
                            pa_t[:, o + k * 128:o + (k + 1) * 128],
                            pa_t[:, OW1 + k * HID:OW1 + (k + 1) * HID],
                            start=(k == 0), stop=(k == KT - 1),
                        )
                    h = keep.tile([128, HID], bf16, tag="hsb", bufs=3)
                    nc.vector.tensor_copy(h[:], ps[:])
                    nc.vector.tensor_copy(ag_sb[:, t, :], ps[:])
                    for half in range(2):
                        pt = pmm.tile([128, HID], f32, tag="mm")
                        ptb = pt[:].bitcast(bf16)[:, 0:128]
                        last_pa_inst = nc.tensor.transpose(
                            ptb, h[:, 128 * half:128 * (half + 1)], ident)
                        nc.vector.tensor_copy(
                            hT[:, half, 128 * t:128 * (t + 1)], ptb)
                    for ci, (lo, hi) in enumerate(CH):
                        if t == hi - 1:
                            nc.gpsimd.dma_start(
                                ag_ins[ci][:].rearrange(
                                    "p (a m) -> p a m", a=hi - lo),
                                ag_sb[:, lo:hi, :])

            # ---- phase B: AllGather h0 (fp8) in 5 chunks, then unpack ----
            with nc.named_scope("allgather"):
                for ci in range(NCH):
                    nc.gpsimd.collective_compute(
                        "AllGather", mybir.AluOpType.bypass,
                        replica_groups=[list(range(NCORES))],
                        ins=[ag_ins[ci].opt()], outs=[ag_outs[ci].opt()],
                    )
                # unpack per 2-tile super group so the SpMM can start on the
                # first supers while the rest of the chunk unpacks
                for ci, (lo, hi) in enumerate(CH):
                    for s in range(lo // 2, hi // 2):
                        o = (2 * s - lo) * HID
                        nc.sync.dma_start(
                            h0a[:, :, 2 * s:2 * s + 2, :],
                            ag_outs[ci][:, o:o + 2 * HID].rearrange(
                                "(r p) (t m) -> p r t m", p=128, t=2))

            # ---- phase D1: out += bias + h0-part (runs in the AG gap) ----
            last_d1_inst = None
            with nc.named_scope("out_gemm"):
                for t in range(NT):
                    ps = pmm.tile([128, OUT_C], f32, tag="mm")
                    nc.tensor.matmul(ps[:], wout_sb[0:1, OO:OO + 128],
                                     wout_sb[0:1, OB:OB + OUT_C],
                                     start=True, stop=False)
                    for k in range(2):
                        last_d1_inst = nc.tensor.matmul(
                            ps[:],
                            hT[:, k, 128 * t:128 * (t + 1)],
                            wout_sb[:, OWO + k * OUT_C:OWO + (k + 1) * OUT_C],
                            start=False, stop=(k == 1),
                        )
                    nc.vector.tensor_copy(o_sb[:, t, :], ps[:])

            # ---- phase C: SpMM, fp8 DoubleRow, A moving / h0 stationary ----
            # out[feat, dest] += sum_src h0a[src, feat] * A[src, dest]
            # One ldweights per (super, feat-half) shared by the 3 dest-chunk
            # matmuls x 1 adjacency; the whole PE stream is chained so no
            # other weight load can slip inside a reuse group.
            chain_prev = [None]

            def chain(bi, extra=None):
                if chain_prev[0] is not None:
                    add_dep_helper(bi.ins, chain_prev[0].ins,
                                   reason="pe weight-reuse chain")
                if extra is not None:
                    add_dep_helper(bi.ins, extra.ins, reason="pe chain head")
                chain_prev[0] = bi

            with nc.named_scope("spmm"):
                for a in range(2):
                    ps6 = pss.tile([128, 2, 3, 512], f32, tag="spmm",
                                   name=f"ps_spmm{a}")
                    for p in range(NG):
                        g = G_ORDER[p]
                        r, tl = g // (NT // 2), 2 * (g % (NT // 2))
                        at = keep.tile([128, 2, DCOLS], f8, tag="a", bufs=36)
                        off = (a * NG + p) * 2 * DCOLS
                        nc.scalar.dma_start(
                            at[:], A_d[:, off:off + 2 * DCOLS]
                            .rearrange("p (a m) -> p a m", a=2))
                        for fh in range(2):
                            lhs = h0a[:, r, tl:tl + 2,
                                      128 * fh:128 * (fh + 1)]
                            head = (a == 0 and p == 0 and fh == 0)
                            dpos = 0
                            for d, w in enumerate(DCH):
                                mm = nc.tensor.matmul(
                                    ps6[:, fh, d, 0:w], lhs,
                                    at[:, :, dpos:dpos + w],
                                    start=(p == 0), stop=(p == NG - 1),
                                    perf_mode=DR,
                                )
                                # head of chain waits for all pre-spmm PE work
                                chain(mm, extra=last_pa_inst
                                      if (head and d == 0) else None)
                                if head and d == 0:
                                    add_dep_helper(mm.ins, last_d1_inst.ins,
                                                   reason="pe chain head d1")
                                dpos += w
                    nc.vector.tensor_copy(
                        hT[:, 2 + 2 * a, 0:DCOLS],
                        ps6[:, 0].rearrange("p a b -> p (a b)")[:, 0:DCOLS])
                    nc.scalar.copy(
                        hT[:, 2 + 2 * a + 1, 0:DCOLS],
                        ps6[:, 1].rearrange("p a b -> p (a b)")[:, 0:DCOLS])
                    if a == 0:
                        # ---- phase D2: out += h1-part (chained into the
                        # PE stream between the two adjacency passes) ----
                        with nc.named_scope("out_gemm"):
                            for t in range(NT):
                                rows = min(128, ROWS - 128 * t)
                                ps = pmm.tile([128, OUT_C], f32, tag="mm")
                                for k in range(2, 4):
                                    mmo = nc.tensor.matmul(
                                        ps[:rows],
                                        hT[:, k, 128 * t:128 * t + rows],
                                        wout_sb[:, OWO + k * OUT_C:
                                                OWO + (k + 1) * OUT_C],
                                        start=(k == 2), stop=(k == 3),
                                    )
                                    chain(mmo)
                                nc.vector.tensor_add(
                                    o_sb[:rows, t, :], o_sb[:rows, t, :],
                                    ps[:rows])

            # ---- phase D3: out += h2-part, cast bf16, single DMA out ----
            with nc.named_scope("out_gemm"):
                for t in range(NT):
                    rows = min(128, ROWS - 128 * t)
                    ps = pmm.tile([128, OUT_C], f32, tag="mm")
                    for k in range(4, 6):
                        nc.tensor.matmul(
                            ps[:rows],
                            hT[:, k, 128 * t:128 * t + rows],
                            wout_sb[:, OWO + k * OUT_C:OWO + (k + 1) * OUT_C],
                            start=(k == 4), stop=(k == 5),
                        )
                    ob = keep.tile([128, OUT_C], bf16, tag="ob", bufs=4)
                    nc.vector.tensor_add(ob[:rows], ps[:rows],
                                         o_sb[:rows, t, :])
                    nc.sync.dma_start(out[128 * t:128 * t + rows, :],
                                      ob[:rows, :])
    _dedup_ldweights(nc)
    nc.compile()
    return nc


def kernel(x, adj1_rows, adj1_cols, adj1_vals, adj2_rows, adj2_cols, adj2_vals,
           W1, W_out, b_out):
    global LAST_EXEC_NS, LAST_RESULTS
    _install_trace_shim()
    x = np.asarray(x, np.float32)
    W1 = np.ascontiguousarray(np.asarray(W1, np.float32))
    W_out = np.ascontiguousarray(np.asarray(W_out, np.float32)).copy()
    b_out = np.asarray(b_out, np.float32).ravel()
    # undo the fp8-range pre-scaling of A1/A2 in the matching W_out rows
    W_out[HID:2 * HID] *= 1.0 / 16.0
    W_out[2 * HID:3 * HID] *= 1.0 / 32.0

    w1_cols = W1.reshape(KT, 128, HID).transpose(1, 0, 2).reshape(128, KT * HID)
    blob_b = np.zeros((128, BLOBB), np.float32)
    blob_b[:, OWO:OWO + 6 * OUT_C] = \
        W_out.reshape(6, 128, OUT_C).transpose(1, 0, 2).reshape(128, 6 * OUT_C)
    blob_b[0, OB:OB + OUT_C] = b_out
    blob_b[0, OO:OO + 128] = 1.0
    blob_b[:, OI:OI + 128] = np.eye(128, dtype=np.float32)
    blob_b = blob_b.astype(bfnp)

    a1r = np.asarray(adj1_rows, np.int64)
    a1c = np.asarray(adj1_cols, np.int64)
    a1v = np.asarray(adj1_vals, np.float32)
    a2r = np.asarray(adj2_rows, np.int64)
    a2c = np.asarray(adj2_cols, np.int64)
    a2v = np.asarray(adj2_vals, np.float32)

    in_maps = []
    for c in range(NCORES):
        xtp = np.zeros((IN_C, PROWS), np.float32)
        xtp[:, :ROWS] = x[c * ROWS:(c + 1) * ROWS].T
        blob_a = np.concatenate([
            w1_cols,
            xtp.reshape(KT, 128, NT, 128).transpose(1, 2, 0, 3)
            .reshape(128, NT * KT * 128),
        ], axis=1).astype(bfnp)
        A_pack = np.concatenate([
            _pack_adj(a1r, a1c, a1v, c, 16.0),
            _pack_adj(a2r, a2c, a2v, c, 32.0),
        ], axis=1)
        in_maps.append({"blob_a": blob_a, "blob_b": blob_b, "A_d": A_pack})

    nc = _build()
    try:
        res = bass_utils.run_bass_kernel_spmd(
            nc, in_maps, core_ids=list(range(NCORES)), trace=True,
            trace_cores=[0])
    except Exception:
        res = bass_utils.run_bass_kernel_spmd(
            nc, in_maps, core_ids=list(range(NCORES)), trace=False)
    LAST_EXEC_NS = res.exec_time_ns
    LAST_RESULTS = res
    return np.concatenate(
        [np.asarray(res.results[c]["out"]).astype(np.float32)
         for c in range(NCORES)], axis=0)
